# revision 4
# baseline (speedup 1.0000x reference)
"""LRU (linear recurrent unit) Trainium2 kernel.

h_t = lam * h_{t-1} + gam * x_t  per channel, lam = exp(-exp(nu_logs)),
gam = sqrt(1 - lam^2).  Uses h = gam * s with s_t = lam*s_{t-1} + x_t so the
gamma scale happens once on the scalar engine after the scan.

Sharding (per the b*d-parallel recurrence structure): 8 cores = 8 channel
groups of 128 channels, each core scans all 4 batches over the full 8192
sequence.  Host-side sharding lays each core's slice out channel-major
([128, B, I]) so every DMA is the canonical contiguous-per-partition
pattern; the gather transposes back.  No cross-core communication.

HBM I/O is fp16 (the 2e-2 gate leaves ~20x margin): host downcasts x, the
device computes in f32, and the scalar engine fuses the gamma scale with the
fp16 downcast of the output.

The DVE TensorTensorScan runs its affine recurrence at ~2 cycles/element
(feedback crosses two ALU stages), which made the plain scan the bottleneck
(70us/core > the 47us fp16 DMA floor).  So the scan is radix-2 decimated:

    t_k      = lam * x_{2k}                    (Pool, broadcast mult)
    y_k      = t_k + x_{2k+1}                  (Pool, add)
    s_{2k+1} = lam^2 * s_{2k-1} + y_k          (DVE scan, half the columns)
    s_{2k}   = lam * s_{2k-1} + x_{2k}         (DVE scalar_tensor_tensor)
    h_t      = gam * s_t                       (ACT, fused fp16 downcast)

which halves the serial-scan columns; the elementwise passes run at
~1 cyc/elem on Pool (GpSimd) / DVE / ACT.  (scalar_tensor_tensor is
DVE-only on TRN2 — the Pool engine rejects TensorScalarPtr — so the pair
compress is two plain tensor_tensor ops there.)  Per-batch odd states land
in a contiguous fp16 "strip" tile at offset 1 (col 0 = 0), so the shifted
odd-state read for the even reconstruct is a plain sub-AP, no carry copies;
the scan keeps fp32 state internally and chains through the fp16 strip.

Loads are issued on the SP HWDGE ring, stores on the ACT ring so stores
never block load prefetch.
"""

import numpy as np
from contextlib import ExitStack

import concourse.bass as bass
import concourse.tile as tile
from concourse import bacc, mybir
from concourse.bass_utils import run_bass_kernel_spmd

B, I, D = 4, 8192, 1024
P = 128             # channels per core = SBUF partitions
C = 4096            # seq steps per chunk (fp16 load tile)
C2 = C // 2         # pair columns per chunk
NCHUNK = I // C     # chunks per batch
I2 = I // 2         # pair columns per batch

F32 = mybir.dt.float32
F16 = mybir.dt.float16

MULT = mybir.AluOpType.mult
ADD = mybir.AluOpType.add


def _lru_kernel(ctx: ExitStack, tc: tile.TileContext, xs, nu, ys):
    nc = tc.nc
    const = ctx.enter_context(tc.tile_pool(name="const", bufs=1))
    xpool = ctx.enter_context(tc.tile_pool(name="x", bufs=3))
    tpool = ctx.enter_context(tc.tile_pool(name="t", bufs=3))
    ypool = ctx.enter_context(tc.tile_pool(name="y", bufs=3))
    strip = ctx.enter_context(tc.tile_pool(name="strip", bufs=2))
    epool = ctx.enter_context(tc.tile_pool(name="se", bufs=3))
    hpool = ctx.enter_context(tc.tile_pool(name="h", bufs=3))

    # --- per-channel decay lam, lam^2, and input scale gam, [P, 1] ---
    nu1 = const.tile([P, 1], F32)
    nc.sync.dma_start(out=nu1[:], in_=nu.rearrange("(p o) -> p o", o=1))
    nus = const.tile([P, 1], F32)
    nc.scalar.activation(nus[:], nu1[:], mybir.ActivationFunctionType.Exp)
    lam = const.tile([P, 1], F32)
    nc.scalar.activation(lam[:], nus[:], mybir.ActivationFunctionType.Exp,
                         scale=-1.0)
    lam2 = const.tile([P, 1], F32)
    nc.vector.tensor_mul(lam2[:], lam[:], lam[:])
    gam = const.tile([P, 1], F32)
    nc.scalar.activation(gam[:], lam2[:], mybir.ActivationFunctionType.Sqrt,
                         scale=-1.0, bias=1.0)

    for b in range(B):
        # odd-state strip: col 0 = s_{-1} = 0, col 1+j = s_{2j+1}
        s_odd = strip.tile([P, I2 + 1], F16)
        nc.gpsimd.memset(s_odd[:, 0:1], 0.0)
        for i in range(NCHUNK):
            x_t = xpool.tile([P, C], F16)
            nc.sync.dma_start(out=x_t[:], in_=xs[:, b, i * C:(i + 1) * C])

            # pair compress on Pool: y = lam*x_even + x_odd (two TT ops)
            t_t = tpool.tile([P, C2], F16)
            nc.gpsimd.tensor_tensor(out=t_t[:],
                                    in0=lam[:, 0:1].broadcast_to([P, C2]),
                                    in1=x_t[:, 0::2], op=MULT)
            y_t = ypool.tile([P, C2], F16)
            nc.gpsimd.tensor_tensor(out=y_t[:], in0=t_t[:],
                                    in1=x_t[:, 1::2], op=ADD)

            # odd-state scan at lam^2 on DVE, into the strip at offset 1
            lo = i * C2
            nc.vector.tensor_tensor_scan(
                out=s_odd[:, 1 + lo:1 + lo + C2],
                data0=lam2[:, 0:1].broadcast_to([P, C2]),
                data1=y_t[:],
                initial=0.0 if i == 0 else s_odd[:, lo:lo + 1],
                op0=MULT,
                op1=ADD,
            )

            # even reconstruct on DVE: s_even = lam*s_{2k-1} + x_even
            s_ev = epool.tile([P, C2], F16)
            nc.vector.scalar_tensor_tensor(
                out=s_ev[:],
                in0=s_odd[:, lo:lo + C2],
                scalar=lam[:, 0:1],
                in1=x_t[:, 0::2],
                op0=MULT,
                op1=ADD,
            )

            # gamma scale + fp16 downcast on ACT, interleaved into h
            h_t = hpool.tile([P, C], F16)
            nc.scalar.activation(h_t[:, 0::2], s_ev[:],
                                 mybir.ActivationFunctionType.Copy,
                                 scale=gam[:, 0:1])
            nc.scalar.activation(h_t[:, 1::2], s_odd[:, 1 + lo:1 + lo + C2],
                                 mybir.ActivationFunctionType.Copy,
                                 scale=gam[:, 0:1])
            # store on the ACT HWDGE ring; loads stay on the SP ring
            nc.scalar.dma_start(out=ys[:, b, i * C:(i + 1) * C], in_=h_t[:])


def _build_nc(num_devices=8):
    nc = bacc.Bacc("TRN2", target_bir_lowering=False, debug=False,
                   num_devices=num_devices)
    xs = nc.dram_tensor("xs", [P, B, I], F16, kind="ExternalInput").ap()
    nu = nc.dram_tensor("nu", [P], F32, kind="ExternalInput").ap()
    ys = nc.dram_tensor("ys", [P, B, I], F16, kind="ExternalOutput").ap()
    with tile.TileContext(nc) as tc:
        with ExitStack() as ctx:
            _lru_kernel(ctx, tc, xs, nu, ys)
    nc.compile()
    return nc


_NC = None


def _build():
    global _NC
    if _NC is None:
        _NC = _build_nc()
    return _NC


def _in_maps(x, nu_logs):
    # x: [B, I, D] -> per core c: [P, B, I] slice of channels (host-side
    # shard + layout change so device DMAs are contiguous per partition;
    # fp16 downcast here halves device HBM traffic)
    xt = np.transpose(x, (2, 0, 1)).astype(np.float16)  # [D, B, I]
    maps = []
    for c in range(8):
        maps.append({
            "xs": xt[c * P:(c + 1) * P],
            "nu": np.ascontiguousarray(nu_logs[c * P:(c + 1) * P],
                                       dtype=np.float32),
        })
    return maps


def kernel(x, nu_logs, _trace=False, **_tk):
    x = np.asarray(x, dtype=np.float32)
    nu_logs = np.asarray(nu_logs, dtype=np.float32)
    nc = _build()
    r = run_bass_kernel_spmd(nc, _in_maps(x, nu_logs), list(range(8)),
                             trace=_trace, **_tk)
    out = np.empty((D, B, I), np.float16)
    for c in range(8):
        out[c * P:(c + 1) * P] = r.results[c]["ys"]
    out = np.transpose(out, (1, 2, 0)).astype(np.float32)  # [B, I, D]
    if _trace:
        return out, r
    return out
